# revision 80
# baseline (speedup 1.0000x reference)
"""Trainium2 Bass kernel for BaseLUTLayer (probabilistic LUT node eval).

Math (per reference):
  x_eff = where(flip, 1 - x, x)                      # (B, IN)
  g[b,n,j] = x_eff[b, mapping[n,j]]                  # gather, (B, N, 6)
  out[b,n] = sum_k sigmoid(lut[n,k]) * prod_j (g_j if bit_j(k) else 1-g_j)

Evaluated on-device in the MONOMIAL (Moebius) basis: the sigmoid table is
transformed once per node into multilinear-polynomial coefficients c
(6 in-place butterfly subtracts), after which every fold level is a pure
FMA with NO subtraction:
  level 0:   V0[q] = c[2q] + a0 * c[2q+1]     (tensor_scalar FMA,
             per-partition fp32 scalar pair; split across ACT/Pool/DVE)
  level 1-5: V[r]  = V[2r]  + a_j * V[2r+1]   (tensor_tensor mul+add, DVE)

Flip is applied per gathered row as a_j = |flip - x| (exact for flip in
{0,1}): Pool tensor_sub for the subtract, ACT Abs.

Sharding: nodes split 8 ways (1024 nodes/core); batch replicated.
x and flip are host-transposed to (IN, B) so dma_gather (the SWDGE
embedding-lookup primitive) can fetch one row per (node, fanin) index.
Per-core output is (1024, 256) fp16; host concatenates, transposes and
casts to fp32.
"""

import numpy as np

B = 256
IN = 8192
NN = 8192
FAN = 6
NPAT = 64
NCORES = 8
PT = 128  # nodes per tile (partition dim)

_CACHE = {}


def _build_nc(nl, b, inp, fp16=True):
    """Build + compile the SPMD Bass program for one core's slice."""
    import concourse.bacc as bacc
    import concourse.mybir as mybir
    from concourse.tile import TileContext
    from concourse._compat import get_trn_type

    dt = mybir.dt
    Alu = mybir.AluOpType
    Act = mybir.ActivationFunctionType

    nt = nl // PT
    n_idx = nl * FAN          # gather indices total
    n_idx_t = PT * FAN        # per tile (768)
    iw = n_idx // 16          # idx wrap columns

    nc = bacc.Bacc(
        get_trn_type() or "TRN2",
        target_bir_lowering=False,
        debug=False,
        num_devices=NCORES,
    )
    # merged gather table: per input row, 2*b bytes of fp16 x then b bytes of u8 flip
    rowb = 3 * b
    xfT = nc.dram_tensor("xfT", [inp, rowb], dt.uint8, kind="ExternalInput")
    # host-pretransposed to partition-major [128, nt, 64] so the load is one
    # contiguous 2KB burst per partition (strided loads gate the ramp)
    lut = nc.dram_tensor("lut", [128, nt * NPAT], dt.float32, kind="ExternalInput")
    idx = nc.dram_tensor("idx", [128, iw], dt.int16, kind="ExternalInput")
    outT = nc.dram_tensor("outT", [nl, b], dt.float16, kind="ExternalOutput")

    cdt = dt.float16 if fp16 else dt.float32

    # L0 engine split per tile (32 tensor_scalar FMAs): (n_dve, n_act, n_pool).
    # Ramp-aware: tile 0 is DVE-heavy (DVE would otherwise idle while the
    # pipeline fills); steady state balances ACT/Pool/DVE makespans.
    def l0_homes(t):
        if t == nt - 1:
            return (0, 19, 13)
        if t == 0:
            return (27, 5, 0)
        if t == 1:
            return (11, 16, 5)
        if t == 2:
            return (5, 19, 8)
        return (3, 19, 10)

    def adds_on_pool(t, j):
        return False

    with TileContext(nc) as tc:
        with (
            tc.tile_pool(name="const", bufs=1) as cpool,
            tc.tile_pool(name="ld", bufs=3) as ld,
            tc.tile_pool(name="small", bufs=4) as sm,
            tc.tile_pool(name="work", bufs=2) as wk,
        ):
            # --- whole-slice node table prep: sigmoid + Moebius butterflies.
            # Split into groups (tiles 0-1 first) so tile 0's coefficients
            # are ready quickly and don't gate the pipeline ramp. ---
            # idx split: tile 0's gather indices land first so the first
            # gather (the ramp-critical chain) isn't queued behind bulk DMAs
            i0w = n_idx_t // 16
            idx_sb = cpool.tile([128, iw], dt.int16)
            lut_sb = cpool.tile([128, nt, NPAT], dt.float32)
            nc.sync.dma_start(idx_sb[:, 0:i0w], idx[:, 0:i0w])
            nc.sync.dma_start(
                lut_sb[:, :, :],
                lut[:, :].rearrange("p (t k) -> p t k", k=NPAT),
            )
            nc.sync.dma_start(idx_sb[:, i0w:], idx[:, i0w:])

            c = cpool.tile([128, nt, NPAT], dt.float32)
            for t_lo, t_hi in ((0, 2), (2, nt)):
                nc.scalar.activation(
                    c[:, t_lo:t_hi, :], lut_sb[:, t_lo:t_hi, :], Act.Sigmoid
                )
                # all butterflies run on DVE (idle at kernel start; Pool is
                # busy with gather descriptor generation)
                eng = nc.vector
                for j in range(6):
                    w = 1 << j
                    cv = c[:, t_lo:t_hi, :].rearrange(
                        "p t (g two w) -> p (t g) two w", two=2, w=w
                    )
                    # in-place butterfly: odds -= evens
                    eng.tensor_sub(
                        cv[:, :, 1:2, :], cv[:, :, 1:2, :], cv[:, :, 0:1, :]
                    )

            # gather groups: first two tiles arrive solo (shorter latency into
            # the pipeline ramp), the rest in pairs (halved SWDGE fixed cost)
            # one gather per tile: 768 descriptors; larger gathers overflow
            # the SWDGE descriptor ring on real hardware (execution fault)
            groups = [(t,) for t in range(nt)]
            for grp in groups:
                gn = len(grp)
                g = ld.tile([128, gn * FAN, rowb], dt.uint8, tag=f"g{gn}")
                i0 = grp[0] * (n_idx_t // 16)
                nc.gpsimd.dma_gather(
                    g[:, :, :], xfT[:, :],
                    idx_sb[:, i0:i0 + gn * n_idx_t // 16],
                    gn * n_idx_t, gn * n_idx_t, rowb,
                )
                for th, t in enumerate(grp):
                    xg = g[:, th * FAN:(th + 1) * FAN, 0:2 * b].bitcast(dt.float16)
                    fg = g[:, th * FAN:(th + 1) * FAN, 2 * b:rowb]

                    # --- flip: a_j = |f - x| (exact for f in {0,1}) ---
                    # fanin 0 first (short critical path into L0), 1-5 after.
                    # Tile 0's fanin-0 path runs entirely on DVE (fewer
                    # cross-engine hops on the ramp-critical chain).
                    dfx = sm.tile([128, FAN, b], cdt, tag="dfx")
                    xe = sm.tile([128, FAN, b], cdt, tag="xe")
                    if t == 0:
                        nc.vector.scalar_tensor_tensor(
                            dfx[:, 0:1, :], fg[:, 0:1, :], 1.0, xg[:, 0:1, :],
                            Alu.bypass, Alu.subtract,
                        )
                        nc.scalar.activation(xe[:, 0:1, :], dfx[:, 0:1, :], Act.Abs)
                    else:
                        nc.gpsimd.tensor_sub(
                            dfx[:, 0:1, :], fg[:, 0:1, :], xg[:, 0:1, :]
                        )
                        nc.scalar.activation(xe[:, 0:1, :], dfx[:, 0:1, :], Act.Abs)
                    nc.gpsimd.tensor_sub(
                        dfx[:, 1:, :], fg[:, 1:, :], xg[:, 1:, :]
                    )
                    nc.scalar.activation(xe[:, 1:, :], dfx[:, 1:, :], Act.Abs)

                    # --- level 0: V0[q] = c[2q] + a0*c[2q+1], 32 per-q FMAs ---
                    n_dve, n_act, n_pool = l0_homes(t)
                    a0 = xe[:, 0, :]
                    V0 = wk.tile([128, 32, b], cdt, tag="V0")
                    for q in range(32):
                        co = c[:, t, 2 * q + 1:2 * q + 2]
                        ce = c[:, t, 2 * q:2 * q + 1]
                        if q < n_act:
                            nc.scalar.activation(
                                V0[:, q, :], a0, Act.Identity, scale=co, bias=ce
                            )
                        elif q < n_act + n_pool:
                            nc.gpsimd.tensor_scalar(
                                out=V0[:, q, :], in0=a0, scalar1=co, scalar2=ce,
                                op0=Alu.mult, op1=Alu.add,
                            )
                        else:
                            nc.vector.tensor_scalar(
                                out=V0[:, q, :], in0=a0, scalar1=co, scalar2=ce,
                                op0=Alu.mult, op1=Alu.add,
                            )

                    # --- levels 1-5: V = V_even + a_j * V_odd (muls on DVE;
                    # some adds ride Pool STT in steady state) ---
                    V = V0
                    for j in range(1, 6):
                        h = 32 >> j
                        a = xe[:, j:j + 1, :].broadcast_to([128, h, b])
                        P = wk.tile([128, h, b], cdt, tag=f"P{j}")
                        nc.vector.tensor_mul(P[:, :, :], V[:, 1::2, :], a)
                        Vn = wk.tile([128, h, b], cdt, tag=f"V{j}")
                        if adds_on_pool(t, j):
                            nc.gpsimd.scalar_tensor_tensor(
                                Vn[:, :, :], P[:, :, :], 1.0, V[:, 0::2, :],
                                Alu.bypass, Alu.add,
                            )
                        else:
                            nc.vector.tensor_add(Vn[:, :, :], P[:, :, :], V[:, 0::2, :])
                        V = Vn

                    nc.sync.dma_start(outT[t * PT:(t + 1) * PT, :], V[:, 0, :])

    nc.compile()
    return nc


def _prep_core_inputs(x, lut_table, mapping, flip_mask, nl, b, inp, n_cores=NCORES):
    """Host-side layout prep (pure data movement): transpose + slice + index pack."""
    xf = np.empty((inp, 3 * b), np.uint8)                          # (IN, 3B)
    xf[:, :2 * b] = np.ascontiguousarray(x.T, dtype=np.float16).view(np.uint8)
    xf[:, 2 * b:] = np.ascontiguousarray(flip_mask.T).astype(np.uint8)
    nt = nl // PT
    in_maps = []
    for c in range(n_cores):
        sl = slice(c * nl, (c + 1) * nl)
        # partition-major relayout: lut_c[p, t*64+k] = lut[t*128+p, k]
        lut_c = np.ascontiguousarray(
            np.asarray(lut_table[sl], dtype=np.float32)
            .reshape(nt, PT, NPAT)
            .transpose(1, 0, 2)
            .reshape(PT, -1)
        )
        m_c = np.asarray(mapping[sl])                              # (nl, 6) int32
        # gather order: j = (t*6+f)*128 + p  ->  m_c[t*128+p, f]
        order = m_c.reshape(nt, PT, FAN).transpose(0, 2, 1).reshape(-1)
        idx16 = order.astype(np.int16)
        wrapped = np.ascontiguousarray(idx16.reshape(-1, 16).T)    # (16, nl*6/16)
        idx_full = np.tile(wrapped, (8, 1))                        # (128, ...)
        in_maps.append({"xfT": xf, "lut": lut_c, "idx": idx_full})
    return in_maps


def _run(nc, in_maps, **kw):
    from concourse.bass_utils import run_bass_kernel_spmd

    last = None
    for attempt in range(3):
        try:
            return run_bass_kernel_spmd(nc, in_maps, list(range(NCORES)), **kw)
        except Exception as e:  # transient device errors happen on this fabric
            last = e
            if "UNRECOVERABLE" not in str(e) and "UNAVAILABLE" not in str(e):
                raise
    raise last


def kernel(x, lut_table, mapping, flip_mask):
    b, inp = x.shape
    nn = lut_table.shape[0]
    nl = nn // NCORES
    key = (nl, b, inp)
    if key not in _CACHE:
        _CACHE[key] = _build_nc(nl, b, inp)
    nc = _CACHE[key]
    in_maps = _prep_core_inputs(x, lut_table, mapping, flip_mask, nl, b, inp)
    res = _run(nc, in_maps)
    outT = np.concatenate([res.results[c]["outT"] for c in range(NCORES)], axis=0)
    return np.ascontiguousarray(outT.T, dtype=np.float32)
